# revision 49
# baseline (speedup 1.0000x reference)
"""Trainium2 Bass kernel for CRATE-style subspace attention (nn_Attention_37091337568712).

Reference computation (fp32):
    w = x @ Wqkv                    # (b, n, 1024), shared q=k=v projection
    w -> (b, h=16, n, d=64)
    S = (w @ w^T) * d^-0.5          # per head, (b, h, n, n)
    attn = softmax(S, axis=-1) * (1 - mask[:, None, None, :])
    out = attn @ w                  # (b, h, n, d)
    y = out.reshape(b, n, 1024) @ Wout + bout

Sharding: 8 cores = 2 batches x 4 head-groups (4 heads each). Each core
computes its 4 heads end-to-end including a partial output projection
(Wout rows for its heads); host sums the 4 partials per batch (the
"all-reduce" of the output projection) and adds bout.

Device kernel (per core) highlights:
  - softmax without max-subtraction (S*scale ~ N(0,1), exp is safe in fp32)
  - post-softmax column mask folded into V (V' = (1-mask_j) * w_j); an
    unmasked ones column in V' makes row 64 of the AV accumulator the
    softmax denominator for free
  - i-half (ibh) outer loop: each half's AV output is normalized and fed
    to the output projection while the next half's S/exp/AV stream runs,
    so the tail after the last attention matmul is short
  - 1/den row -> all-partitions broadcast via a stride-0 DMA (no PE
    broadcast matmuls)
  - output projection matmuls interleaved into the second i-half's jc
    loop as PE filler (PSUM s-tag tiles double as y accumulators)
"""

import sys

if "/opt/trn_rl_repo" not in sys.path:
    sys.path.insert(0, "/opt/trn_rl_repo")

import numpy as np

import concourse.bass as bass
import concourse.mybir as mybir
from concourse import library_config, masks
from concourse.bass_utils import run_bass_kernel_spmd
from concourse.tile import TileContext

FP = mybir.dt.float32
I32 = mybir.dt.int32
F32R = mybir.dt.float32r  # 4x faster PE path than fp32, ~fp32 accuracy
F16 = mybir.dt.float16    # same PE rate, half the power/bytes of f32r


def _r(ap):
    return ap.bitcast(F32R)


def _split_multiwaits(bir_json: bytes) -> bytes:
    """This container's walrus supports a single sync wait per instruction
    (setupSyncWait: 'Too many sync wait commands', seen on the Tile tail
    Drain). Split any multi-wait instruction into a chain of single-wait
    EventSemaphore instructions (same engine, program order) followed by
    the original instruction keeping its last wait."""
    import json

    bir = json.loads(bir_json)
    changed = False
    for fn in bir.get("functions", []):
        for bb in fn.get("blocks", []):
            insts = bb.get("instructions")
            if insts is None:
                continue
            new_insts = []
            for ins in insts:
                si = ins.get("sync_info")
                waits = si.get("on_wait") if si else None
                if waits and len(waits) > 1:
                    changed = True
                    for wi, w in enumerate(waits[:-1]):
                        new_insts.append({
                            "name": f"{ins['name']}_w{wi}",
                            "opcode": "EventSemaphore",
                            "engine": ins["engine"],
                            "ins": [],
                            "outs": [],
                            "debug": ins.get("debug", 0),
                            "sync_info": {"on_wait": [w], "on_update": []},
                        })
                    si["on_wait"] = [waits[-1]]
                new_insts.append(ins)
            bb["instructions"] = new_insts
    if not changed:
        return bir_json
    return json.dumps(bir).encode()


def _install_bir_legalizer():
    from concourse import bass2jax, bass_utils

    if getattr(bass2jax, "_multiwait_legalizer_installed", False):
        return
    orig = bass_utils.compile_bir_kernel

    def wrapped(bir_json, tmpdir, neff_name="file.neff"):
        try:
            return orig(_split_multiwaits(bytes(bir_json)), tmpdir, neff_name)
        except BaseException as e:
            # XLA swallows python exceptions from the compile callback;
            # persist the real error for debugging.
            import subprocess, traceback
            try:
                with open("/tmp/bass_compile_err.txt", "w") as f:
                    traceback.print_exc(file=f)
                    ee = e
                    while ee is not None:
                        if isinstance(ee, subprocess.CalledProcessError):
                            out = ee.stdout or ""
                            if isinstance(out, bytes):
                                out = out.decode(errors="replace")
                            f.write("\n==WALRUS STDOUT (tail)==\n" + out[-12000:])
                        ee = ee.__cause__ or ee.__context__
            except Exception:
                pass
            raise

    bass2jax.compile_bir_kernel = wrapped
    bass2jax._multiwait_legalizer_installed = True

N = 2048          # sequence length
DIM = 1024        # model dim
DH = 64           # head dim
HEADS_PER_CORE = 4
PAIRS = 2         # head pairs per core (2 heads = 128 partitions stacked)
EC = HEADS_PER_CORE * DH   # 256 local inner columns
KC = DIM // 128   # 8 contraction chunks for the projection
JC = N // 128     # 16 key chunks
SCALE = DH ** -0.5

_program_cache = {}


def build_program():
    nc = bass.Bass()

    xT = nc.declare_dram_parameter("xT", [DIM, N], F16, isOutput=False)
    wqkv = nc.declare_dram_parameter("wqkv", [DIM, EC], F16, isOutput=False)
    wout = nc.declare_dram_parameter("wout", [EC, DIM], F16, isOutput=False)
    mask_d = nc.declare_dram_parameter("mask", [N], I32, isOutput=False)
    y = nc.declare_dram_parameter("y", [N, DIM], F16, isOutput=True)
    # DRAM bounce rows for the 1/den broadcast (DMA from DRAM supports a
    # 0-stride partition-broadcast source; SBUF sources do not)
    scr = nc.declare_dram_parameter("scr", [8, 1024], FP, isOutput=True)

    EXPF = mybir.ActivationFunctionType.Exp

    with TileContext(nc) as tc:
        with (
            tc.tile_pool(name="const", bufs=1) as constp,
            tc.tile_pool(name="wts", bufs=1) as wts,
            tc.tile_pool(name="persist", bufs=1) as persist,
            tc.tile_pool(name="xin", bufs=8) as xin,
            tc.tile_pool(name="epool", bufs=6) as epool,
            tc.tile_pool(name="bsb", bufs=2) as bsb,
        ):
            # ---- constants / small inputs ----
            ident = constp.tile([16, 16], FP)
            masks.make_identity(nc, ident[:])
            ident2 = constp.tile([128, 128], F16)
            masks.make_identity(nc, ident2[:])

            # exp logit-shift constant (see the activation below)
            shiftb = constp.tile([128, 1], FP)
            nc.vector.memset(shiftb[:], -10.0)

            mask_i = constp.tile([16, 128], I32)
            nc.sync.dma_start(mask_i[:], mask_d.rearrange("(a b) -> a b", a=16))
            mask_f = constp.tile([16, 128], FP)
            # 1 - mask, cast int32 -> fp32
            nc.vector.tensor_scalar(
                out=mask_f[:], in0=mask_i[:], scalar1=-1.0, scalar2=1.0,
                op0=mybir.AluOpType.mult, op1=mybir.AluOpType.add,
            )

            # ---- persistent big tiles ----
            wTh = persist.tile([128, PAIRS, N], F16)       # [d2, pair, i]
            v2 = persist.tile([128, PAIRS, JC, 224], F16)  # [j, pair, jc, d2+ones+pad]
            osT2 = persist.tile([128, PAIRS, N], F16)       # scaled attn out, [e, pair, i]
            maskc = persist.tile([128, JC], FP)           # (1-mask) in [j%128, jc]
            # softmax denominators, spread [128, 8 per k] for a cheap batched
            # reciprocal (engine APs may only start at partition 0/32/64/96,
            # and DVE reciprocal costs ~8 cycles per element per lane)
            den_sp = persist.tile([128, 64], FP)
            recip_sp = persist.tile([128, 64], FP)

            # unmasked ones columns (64 and 129): the AV matmul's M=65
            # weight includes them so row 64 of the AV accumulator becomes
            # the (unmasked) softmax denominator for free.
            # pad zeroing on the otherwise-idle gpsimd engine: keeping it
            # off the DVE queue unblocks the startup masking chain
            nc.gpsimd.memset(v2[:, :, :, 130:224], 0.0)
            nc.vector.memset(v2[:, :, :, 64:130:65], 1.0)

            # ---- weights (wqkv chunked so xT chunks start flowing early) ----
            wq_sb = wts.tile([128, KC, EC], F16)
            wq_r = wqkv.rearrange("(kc p) e -> p kc e", p=128)
            wout_sb = wts.tile([128, PAIRS, DIM], F16)

            # ---- phase 1: projection  wTh[d2, i] = Wqkv_cols^T @ x^T ----
            with tc.tile_pool(name="ps_proj", bufs=1, space="PSUM") as ps_proj:
                proj_ps = [ps_proj.tile([128, 512], FP, name=f"proj{t}", tag=f"proj{t}")
                           for t in range(8)]
                xts = []
                dma_engs = [nc.sync, nc.scalar]
                for kc in range(KC):
                    dma_engs[(kc + 1) % 2].dma_start(wq_sb[:, kc, :], wq_r[:, kc, :])
                    xt = xin.tile([128, N], F16, name="xt")
                    # spread the input load across engine DMA rings so the
                    # chunks transfer in parallel instead of serializing on
                    # the SP ring
                    dma_engs[kc % 2].dma_start(xt[:], xT[kc * 128:(kc + 1) * 128, :])
                    xts.append(xt)
                    for pair in range(PAIRS):
                        for rb in range(4):
                            nc.tensor.matmul(
                                proj_ps[pair * 4 + rb][:],
                                wq_sb[:, kc, pair * 128:(pair + 1) * 128],
                                xt[:, rb * 512:(rb + 1) * 512],
                                start=(kc == 0), stop=(kc == KC - 1),
                            )
                nc.scalar.dma_start(wout_sb[:], wout.rearrange("(pc p) m -> p pc m", p=128))
                # split the PSUM->SBUF copies between DVE and the (still
                # idle) scalar engine to halve the startup chain
                COPYF = mybir.ActivationFunctionType.Copy
                # pair0 cols 0:1024 first (they gate the first transposes
                # and the first S matmuls); everything else trails on DVE
                order = [(0, 0), (0, 1), (0, 2), (0, 3),
                         (1, 0), (1, 1), (1, 2), (1, 3)]
                for n_, (pair, rb) in enumerate(order):
                    dst = wTh[:, pair, rb * 512:(rb + 1) * 512]
                    cpsrc = proj_ps[pair * 4 + rb][:]
                    if n_ == 0:
                        nc.scalar.activation(dst, cpsrc, COPYF)
                    else:
                        nc.vector.tensor_copy(dst, cpsrc)

            # HAM medicine: the PE clock-gate re-throttles to 1.2 GHz
            # whenever a ~3.4us activity window sees the PE partly idle,
            # and the cold state then self-sustains through the steady jc
            # loop. Dense back-to-back throwaway matmuls carry the PE
            # through dependency-thin stretches (phase entry, block
            # boundaries) at full occupancy.
            def emit_burst(pool, tag, n, bufs=None):
                wbt = pool.tile([128, 1024], FP, name="wb", tag=tag, bufs=bufs)
                for t in range(n):
                    nc.tensor.matmul(
                        wbt[:, (t % 2) * 512:(t % 2 + 1) * 512],
                        wTh[:, 0, 0:128],
                        wTh[:, 0, (t % 2) * 512:(t % 2 + 1) * 512],
                        start=True, stop=True,
                    )

            # ---- phase 2: transposes (mask layout + V') ----
            with tc.tile_pool(name="ps_tr", bufs=4, space="PSUM") as ps_tr:
                mt_ps = ps_tr.tile([128, 16], FP, tag="mt", bufs=1)
                nc.tensor.transpose(mt_ps[:], mask_f[:], ident[:])
                nc.vector.tensor_copy(maskc[:], mt_ps[:])
                # enter the transpose stream warm
                emit_burst(ps_tr, "wb", 6, bufs=1)

                tr_host = None
                for pair in range(1):
                    for jc in range(JC):
                        q = (pair * JC + jc) % 4
                        if q == 0:
                            # 4 transposes share one PSUM tile: 16 in flight
                            # across 4 banks keeps the PE stream dense
                            tr_host = ps_tr.tile([128, 512], F16, name="tr",
                                                 tag="tr", bufs=4)
                        tslot = tr_host[:, q * 128:(q + 1) * 128]
                        nc.tensor.transpose(
                            tslot, wTh[:, pair, jc * 128:(jc + 1) * 128], ident2[:]
                        )
                        # V' = (1 - mask_j) * w_j, applied per partition (j);
                        # one DVE op covers both heads via a [128, 2, 64] AP
                        nc.vector.tensor_scalar_mul(
                            v2[:, pair, jc, 0:130]
                              .rearrange("p (h x) -> p h x", h=2)[:, :, 0:64],
                            tslot.rearrange("p (h x) -> p h x", h=2),
                            maskc[:, jc:jc + 1],
                        )

            # ---- phase 3+4: attention with fused normalize + out-projection ----
            with (
                tc.tile_pool(name="ps_s", bufs=2, space="PSUM") as ps_s,
                tc.tile_pool(name="ps_av", bufs=2, space="PSUM") as ps_av,
            ):
                def emit_outproj(ic, on_scalar=False):
                    # y[ic] = sum_pair osT2[:, pair, ic]^T @ Wout_pair
                    y_ps = ps_s.tile([128, 1024], FP, name="yp", tag="s")
                    for nb in range(2):
                        for pair in range(PAIRS):
                            nc.tensor.matmul(
                                y_ps[:, nb * 512:(nb + 1) * 512],
                                osT2[:, pair, ic * 128:(ic + 1) * 128],
                                wout_sb[:, pair, nb * 512:(nb + 1) * 512],
                                start=(pair == 0), stop=(pair == PAIRS - 1),
                            )
                    y_sb = bsb.tile([128, 1024], F16, name="ysb", tag="ysb", bufs=3)
                    if on_scalar:
                        nc.scalar.activation(y_sb[:], y_ps[:],
                                             mybir.ActivationFunctionType.Copy)
                    else:
                        nc.vector.tensor_copy(y_sb[:], y_ps[:])
                    nc.sync.dma_start(y[ic * 128:(ic + 1) * 128, :], y_sb[:])

                # fill the PE through the jc-pipeline bootstrap
                emit_burst(ps_s, "s", 7)
                for ibh in range(2):
                    i0 = ibh * 1024
                    for pair in range(PAIRS):
                        av_t = [ps_av.tile([128, 1024], FP, name=f"av{hh}", tag="av")
                                for hh in range(2)]
                        for jc in range(JC):
                            s_t = [ps_s.tile([128, 1024], FP, name=f"s{hh}", tag="s")
                                   for hh in range(2)]
                            for sb in range(2):
                                for hh in range(2):
                                    p0 = hh * 64
                                    nc.tensor.matmul(
                                        s_t[hh][:, sb * 512:(sb + 1) * 512],
                                        wTh[p0:p0 + 64, pair, jc * 128:(jc + 1) * 128],
                                        wTh[p0:p0 + 64, pair,
                                            i0 + sb * 512:i0 + (sb + 1) * 512],
                                        start=True, stop=True,
                                        tile_position=(p0, 0),
                                    )
                            e_t = []
                            for hh in range(2):
                                e = epool.tile([128, 1024], F16, name=f"e{hh}", tag="e")
                                # -10 logit shift: cancels in the softmax
                                # ratio but keeps exp within fp16 range (max
                                # observed logit ~19.8 = diag ||w||^2/8;
                                # fp16 overflows at e^11.09)
                                nc.scalar.activation(e[:], s_t[hh][:], EXPF,
                                                     scale=SCALE, bias=shiftb[:])
                                e_t.append(e)
                            for sb in range(2):
                                for hh in range(2):
                                    nc.tensor.matmul(
                                        av_t[hh][0:128, sb * 512:(sb + 1) * 512],
                                        v2[:, pair, jc, hh * 65:hh * 65 + 128],
                                        e_t[hh][:, sb * 512:(sb + 1) * 512],
                                        start=(jc == 0), stop=(jc == JC - 1),
                                        skip_group_check=True,
                                    )
                            # PE filler: previous i-half's output projection
                            # (also covers the pair-transition exp bubble)
                            if ibh == 1 and jc % 4 == 1:
                                emit_outproj(pair * 4 + (jc - 1) // 4)
                            # each block's first two jc iterations are a
                            # pipeline-refill transient (PE waits on exp in
                            # lockstep) -- pad them to full PE occupancy.
                            # In the very first block the pair1 transposes
                            # are that padding (real work instead of junk).
                            if ibh == 0 and pair == 0 and jc < 6:
                                if jc % 2 == 0:
                                    trh2 = ps_s.tile([128, 1024], FP,
                                                     name="trh2", tag="s")
                                for w_ in range(3):
                                    t_ = jc * 3 + w_
                                    if t_ >= JC:
                                        continue
                                    slot = trh2[:, (jc % 2) * 384 + w_ * 128:
                                                (jc % 2) * 384 + (w_ + 1) * 128]
                                    slot16 = slot.bitcast(F16)[:, 0:128]
                                    nc.tensor.transpose(
                                        slot16,
                                        wTh[:, 1, t_ * 128:(t_ + 1) * 128],
                                        ident2[:],
                                    )
                                    nc.vector.tensor_scalar_mul(
                                        v2[:, 1, t_, 0:130]
                                          .rearrange("p (h x) -> p h x", h=2)[:, :, 0:64],
                                        slot16.rearrange("p (h x) -> p h x", h=2),
                                        maskc[:, t_:t_ + 1],
                                    )
                            elif jc == 0:
                                emit_burst(ps_s, "s", 4)
                            elif jc == 1:
                                emit_burst(ps_s, "s", 3)

                        # ---- normalize this (ibh, pair): entirely PE-free.
                        # Denominator: row 64 of each AV accumulator -> SBUF
                        # -> [128, 8] spread (DMA iterates the out AP
                        # partition-major, i -> (i//8, i%8)) -> batched
                        # reciprocal -> DRAM bounce row -> partition-broadcast
                        # DMA into an SBUF [128, 1024] tile -> one in-place
                        # multiply on the staged raw AV.
                        kp = (ibh * 2 + pair) * 2
                        for hh in range(2):
                            k = kp + hh
                            trow = bsb.tile([1, 1024], FP, name="trow", tag="trow", bufs=4)
                            nc.vector.tensor_copy(trow[:], av_t[hh][64:65, :])
                            nc.sync.dma_start(den_sp[:, k * 8:(k + 1) * 8], trow[:])
                        nc.vector.tensor_copy(
                            osT2[0:64, pair, i0:i0 + 1024],
                            av_t[0][0:64, :],
                        )
                        nc.vector.tensor_copy(
                            osT2[64:128, pair, i0:i0 + 1024],
                            av_t[1][0:64, :],
                        )
                        nc.vector.reciprocal(recip_sp[:, kp * 8:(kp + 2) * 8],
                                             den_sp[:, kp * 8:(kp + 2) * 8])
                        bc = bsb.tile([128, 1024], FP, name="bc", tag="bc", bufs=2)
                        for hh in range(2):
                            k = kp + hh
                            nc.sync.dma_start(scr[k, :],
                                              recip_sp[:, k * 8:(k + 1) * 8])
                            nc.sync.dma_start(bc[hh * 64:(hh + 1) * 64, :],
                                              scr[k, :].partition_broadcast(64))
                        nc.vector.tensor_tensor(
                            out=osT2[:, pair, i0:i0 + 1024],
                            in0=osT2[:, pair, i0:i0 + 1024],
                            in1=bc[:],
                            op=mybir.AluOpType.mult,
                        )
                        # ride the PE through the dependency-thin block
                        # boundary (the next block's S stream is semaphore
                        # lockstepped with exp and leaves the PE ~70% idle
                        # for ~4us, which cold-traps the clock gate)
                        emit_burst(ps_s, "s", 12 if (ibh, pair) == (1, 1) else 4)

                # the second i-half's out-projection
                for ic in range(8, 16):
                    emit_outproj(ic)

    return nc


def get_program():
    if "nc" not in _program_cache:
        _program_cache["nc"] = build_program()
    return _program_cache["nc"]


def make_in_maps(x, mask, Wqkv, Wout):
    xT_b = [np.ascontiguousarray(x[b].T.astype(np.float16)) for b in range(2)]
    Wq16 = Wqkv.astype(np.float16)
    in_maps = []
    for c in range(8):
        b, hg = c // 4, c % 4
        ec = slice(hg * EC, (hg + 1) * EC)
        in_maps.append({
            "xT": xT_b[b],
            "wqkv": np.ascontiguousarray(Wq16[:, ec]),
            "wout": np.ascontiguousarray(Wout[ec, :].astype(np.float16)),
            "mask": np.ascontiguousarray(mask[b]),
        })
    return in_maps


def assemble(results, bout):
    y = np.stack([
        sum(results[b * 4 + g]["y"].astype(np.float32) for g in range(4))
        for b in range(2)
    ])
    return (y + bout[None, None, :]).astype(np.float32)


def kernel(x, mask, Wqkv, Wout, bout):
    _install_bir_legalizer()
    nc = get_program()
    in_maps = make_in_maps(x, mask, Wqkv, Wout)
    res = run_bass_kernel_spmd(nc, in_maps, core_ids=list(range(8)))
    return assemble(res.results, bout)


if __name__ == "__main__":
    nc = build_program()
    print("program built OK")


# revision 50
# speedup vs baseline: 1.0856x; 1.0856x over previous
"""Trainium2 Bass kernel for CRATE-style subspace attention (nn_Attention_37091337568712).

Reference computation (fp32):
    w = x @ Wqkv                    # (b, n, 1024), shared q=k=v projection
    w -> (b, h=16, n, d=64)
    S = (w @ w^T) * d^-0.5          # per head, (b, h, n, n)
    attn = softmax(S, axis=-1) * (1 - mask[:, None, None, :])
    out = attn @ w                  # (b, h, n, d)
    y = out.reshape(b, n, 1024) @ Wout + bout

Sharding: 8 cores = 2 batches x 4 head-groups (4 heads each). Each core
computes its 4 heads end-to-end including a partial output projection
(Wout rows for its heads); host sums the 4 partials per batch (the
"all-reduce" of the output projection) and adds bout.

Device kernel (per core) highlights:
  - softmax without max-subtraction (S*scale ~ N(0,1), exp is safe in fp32)
  - post-softmax column mask folded into V (V' = (1-mask_j) * w_j); an
    unmasked ones column in V' makes row 64 of the AV accumulator the
    softmax denominator for free
  - i-half (ibh) outer loop: each half's AV output is normalized and fed
    to the output projection while the next half's S/exp/AV stream runs,
    so the tail after the last attention matmul is short
  - 1/den row -> all-partitions broadcast via a stride-0 DMA (no PE
    broadcast matmuls)
  - output projection matmuls interleaved into the second i-half's jc
    loop as PE filler (PSUM s-tag tiles double as y accumulators)
"""

import sys

if "/opt/trn_rl_repo" not in sys.path:
    sys.path.insert(0, "/opt/trn_rl_repo")

import numpy as np

import concourse.bass as bass
import concourse.mybir as mybir
from concourse import library_config, masks
from concourse.bass_utils import run_bass_kernel_spmd
from concourse.tile import TileContext

FP = mybir.dt.float32
I32 = mybir.dt.int32
F32R = mybir.dt.float32r  # 4x faster PE path than fp32, ~fp32 accuracy
F16 = mybir.dt.float16    # same PE rate, half the power/bytes of f32r


def _r(ap):
    return ap.bitcast(F32R)


def _split_multiwaits(bir_json: bytes) -> bytes:
    """This container's walrus supports a single sync wait per instruction
    (setupSyncWait: 'Too many sync wait commands', seen on the Tile tail
    Drain). Split any multi-wait instruction into a chain of single-wait
    EventSemaphore instructions (same engine, program order) followed by
    the original instruction keeping its last wait."""
    import json

    bir = json.loads(bir_json)
    changed = False
    for fn in bir.get("functions", []):
        for bb in fn.get("blocks", []):
            insts = bb.get("instructions")
            if insts is None:
                continue
            new_insts = []
            for ins in insts:
                si = ins.get("sync_info")
                waits = si.get("on_wait") if si else None
                if waits and len(waits) > 1:
                    changed = True
                    for wi, w in enumerate(waits[:-1]):
                        new_insts.append({
                            "name": f"{ins['name']}_w{wi}",
                            "opcode": "EventSemaphore",
                            "engine": ins["engine"],
                            "ins": [],
                            "outs": [],
                            "debug": ins.get("debug", 0),
                            "sync_info": {"on_wait": [w], "on_update": []},
                        })
                    si["on_wait"] = [waits[-1]]
                new_insts.append(ins)
            bb["instructions"] = new_insts
    if not changed:
        return bir_json
    return json.dumps(bir).encode()


def _install_bir_legalizer():
    from concourse import bass2jax, bass_utils

    if getattr(bass2jax, "_multiwait_legalizer_installed", False):
        return
    orig = bass_utils.compile_bir_kernel

    def wrapped(bir_json, tmpdir, neff_name="file.neff"):
        try:
            return orig(_split_multiwaits(bytes(bir_json)), tmpdir, neff_name)
        except BaseException as e:
            # XLA swallows python exceptions from the compile callback;
            # persist the real error for debugging.
            import subprocess, traceback
            try:
                with open("/tmp/bass_compile_err.txt", "w") as f:
                    traceback.print_exc(file=f)
                    ee = e
                    while ee is not None:
                        if isinstance(ee, subprocess.CalledProcessError):
                            out = ee.stdout or ""
                            if isinstance(out, bytes):
                                out = out.decode(errors="replace")
                            f.write("\n==WALRUS STDOUT (tail)==\n" + out[-12000:])
                        ee = ee.__cause__ or ee.__context__
            except Exception:
                pass
            raise

    bass2jax.compile_bir_kernel = wrapped
    bass2jax._multiwait_legalizer_installed = True

N = 2048          # sequence length
DIM = 1024        # model dim
DH = 64           # head dim
HEADS_PER_CORE = 4
PAIRS = 2         # head pairs per core (2 heads = 128 partitions stacked)
EC = HEADS_PER_CORE * DH   # 256 local inner columns
KC = DIM // 128   # 8 contraction chunks for the projection
JC = N // 128     # 16 key chunks
SCALE = DH ** -0.5

_program_cache = {}


def build_program():
    nc = bass.Bass()

    xT = nc.declare_dram_parameter("xT", [DIM, N], F16, isOutput=False)
    wqkv = nc.declare_dram_parameter("wqkv", [DIM, EC], F16, isOutput=False)
    wout = nc.declare_dram_parameter("wout", [EC, DIM], F16, isOutput=False)
    mask_d = nc.declare_dram_parameter("mask", [N], I32, isOutput=False)
    y = nc.declare_dram_parameter("y", [N, DIM], F16, isOutput=True)
    # DRAM bounce rows for the 1/den broadcast (DMA from DRAM supports a
    # 0-stride partition-broadcast source; SBUF sources do not)
    scr = nc.declare_dram_parameter("scr", [8, 1024], FP, isOutput=True)

    EXPF = mybir.ActivationFunctionType.Exp

    with TileContext(nc) as tc:
        with (
            tc.tile_pool(name="const", bufs=1) as constp,
            tc.tile_pool(name="wts", bufs=1) as wts,
            tc.tile_pool(name="persist", bufs=1) as persist,
            tc.tile_pool(name="xin", bufs=8) as xin,
            tc.tile_pool(name="epool", bufs=6) as epool,
            tc.tile_pool(name="bsb", bufs=2) as bsb,
        ):
            # ---- constants / small inputs ----
            ident = constp.tile([16, 16], FP)
            masks.make_identity(nc, ident[:])
            ident2 = constp.tile([128, 128], F16)
            masks.make_identity(nc, ident2[:])

            # exp logit-shift constant (see the activation below)
            shiftb = constp.tile([128, 1], FP)
            nc.vector.memset(shiftb[:], -10.0)

            mask_i = constp.tile([16, 128], I32)
            nc.sync.dma_start(mask_i[:], mask_d.rearrange("(a b) -> a b", a=16))
            mask_f = constp.tile([16, 128], FP)
            # 1 - mask, cast int32 -> fp32
            nc.vector.tensor_scalar(
                out=mask_f[:], in0=mask_i[:], scalar1=-1.0, scalar2=1.0,
                op0=mybir.AluOpType.mult, op1=mybir.AluOpType.add,
            )

            # ---- persistent big tiles ----
            wTh = persist.tile([128, PAIRS, N], F16)       # [d2, pair, i]
            v2 = persist.tile([128, PAIRS, JC, 224], F16)  # [j, pair, jc, d2+ones+pad]
            osT2 = persist.tile([128, PAIRS, N], F16)       # scaled attn out, [e, pair, i]
            maskc = persist.tile([128, JC], FP)           # (1-mask) in [j%128, jc]
            # softmax denominators, spread [128, 8 per k] for a cheap batched
            # reciprocal (engine APs may only start at partition 0/32/64/96,
            # and DVE reciprocal costs ~8 cycles per element per lane)
            den_sp = persist.tile([128, 64], FP)
            recip_sp = persist.tile([128, 64], FP)

            # unmasked ones columns (64 and 129): the AV matmul's M=65
            # weight includes them so row 64 of the AV accumulator becomes
            # the (unmasked) softmax denominator for free.
            # pad zeroing on the otherwise-idle gpsimd engine: keeping it
            # off the DVE queue unblocks the startup masking chain
            nc.gpsimd.memset(v2[:, :, :, 130:224], 0.0)
            nc.vector.memset(v2[:, :, :, 64:130:65], 1.0)

            # ---- weights (wqkv chunked so xT chunks start flowing early) ----
            wq_sb = wts.tile([128, KC, EC], F16)
            wq_r = wqkv.rearrange("(kc p) e -> p kc e", p=128)
            wout_sb = wts.tile([128, PAIRS, DIM], F16)

            # ---- phase 1: projection  wTh[d2, i] = Wqkv_cols^T @ x^T ----
            with tc.tile_pool(name="ps_proj", bufs=1, space="PSUM") as ps_proj:
                proj_ps = [ps_proj.tile([128, 512], FP, name=f"proj{t}", tag=f"proj{t}")
                           for t in range(8)]
                xts = []
                dma_engs = [nc.sync, nc.scalar]
                for kc in range(KC):
                    dma_engs[(kc + 1) % 2].dma_start(wq_sb[:, kc, :], wq_r[:, kc, :])
                    xt = xin.tile([128, N], F16, name="xt")
                    # spread the input load across engine DMA rings so the
                    # chunks transfer in parallel instead of serializing on
                    # the SP ring
                    dma_engs[kc % 2].dma_start(xt[:], xT[kc * 128:(kc + 1) * 128, :])
                    xts.append(xt)
                    for pair in range(PAIRS):
                        for rb in range(4):
                            nc.tensor.matmul(
                                proj_ps[pair * 4 + rb][:],
                                wq_sb[:, kc, pair * 128:(pair + 1) * 128],
                                xt[:, rb * 512:(rb + 1) * 512],
                                start=(kc == 0), stop=(kc == KC - 1),
                            )
                nc.scalar.dma_start(wout_sb[:], wout.rearrange("(pc p) m -> p pc m", p=128))
                # split the PSUM->SBUF copies between DVE and the (still
                # idle) scalar engine to halve the startup chain
                COPYF = mybir.ActivationFunctionType.Copy
                for pair in range(PAIRS):
                    for rb in range(4):
                        dst = wTh[:, pair, rb * 512:(rb + 1) * 512]
                        cpsrc = proj_ps[pair * 4 + rb][:]
                        if rb % 2 == 0:
                            nc.scalar.activation(dst, cpsrc, COPYF)
                        else:
                            nc.vector.tensor_copy(dst, cpsrc)

            # HAM medicine: the PE clock-gate re-throttles to 1.2 GHz
            # whenever a ~3.4us activity window sees the PE partly idle,
            # and the cold state then self-sustains through the steady jc
            # loop. Dense back-to-back throwaway matmuls carry the PE
            # through dependency-thin stretches (phase entry, block
            # boundaries) at full occupancy.
            def emit_burst(pool, tag, n, bufs=None):
                wbt = pool.tile([128, 1024], FP, name="wb", tag=tag, bufs=bufs)
                for t in range(n):
                    nc.tensor.matmul(
                        wbt[:, (t % 2) * 512:(t % 2 + 1) * 512],
                        wTh[:, 0, 0:128],
                        wTh[:, 0, (t % 2) * 512:(t % 2 + 1) * 512],
                        start=True, stop=True,
                    )

            # ---- phase 2: transposes (mask layout + V') ----
            with tc.tile_pool(name="ps_tr", bufs=4, space="PSUM") as ps_tr:
                mt_ps = ps_tr.tile([128, 16], FP, tag="mt", bufs=1)
                nc.tensor.transpose(mt_ps[:], mask_f[:], ident[:])
                nc.vector.tensor_copy(maskc[:], mt_ps[:])
                # enter the transpose stream warm
                emit_burst(ps_tr, "wb", 6, bufs=1)

                tr_host = None
                for pair in range(PAIRS):
                    for jc in range(JC):
                        q = (pair * JC + jc) % 4
                        if q == 0:
                            # 4 transposes share one PSUM tile: 16 in flight
                            # across 4 banks keeps the PE stream dense
                            tr_host = ps_tr.tile([128, 512], F16, name="tr",
                                                 tag="tr", bufs=4)
                        tslot = tr_host[:, q * 128:(q + 1) * 128]
                        nc.tensor.transpose(
                            tslot, wTh[:, pair, jc * 128:(jc + 1) * 128], ident2[:]
                        )
                        # V' = (1 - mask_j) * w_j, applied per partition (j);
                        # one DVE op covers both heads via a [128, 2, 64] AP
                        nc.vector.tensor_scalar_mul(
                            v2[:, pair, jc, 0:130]
                              .rearrange("p (h x) -> p h x", h=2)[:, :, 0:64],
                            tslot.rearrange("p (h x) -> p h x", h=2),
                            maskc[:, jc:jc + 1],
                        )

            # ---- phase 3+4: attention with fused normalize + out-projection ----
            with (
                tc.tile_pool(name="ps_s", bufs=2, space="PSUM") as ps_s,
                tc.tile_pool(name="ps_av", bufs=2, space="PSUM") as ps_av,
            ):
                def emit_outproj(ic, on_scalar=False):
                    # y[ic] = sum_pair osT2[:, pair, ic]^T @ Wout_pair
                    y_ps = ps_s.tile([128, 1024], FP, name="yp", tag="s")
                    for nb in range(2):
                        for pair in range(PAIRS):
                            nc.tensor.matmul(
                                y_ps[:, nb * 512:(nb + 1) * 512],
                                osT2[:, pair, ic * 128:(ic + 1) * 128],
                                wout_sb[:, pair, nb * 512:(nb + 1) * 512],
                                start=(pair == 0), stop=(pair == PAIRS - 1),
                            )
                    y_sb = bsb.tile([128, 1024], F16, name="ysb", tag="ysb", bufs=3)
                    if on_scalar:
                        nc.scalar.activation(y_sb[:], y_ps[:],
                                             mybir.ActivationFunctionType.Copy)
                    else:
                        nc.vector.tensor_copy(y_sb[:], y_ps[:])
                    nc.sync.dma_start(y[ic * 128:(ic + 1) * 128, :], y_sb[:])

                # fill the PE through the jc-pipeline bootstrap
                emit_burst(ps_s, "s", 7)
                for ibh in range(2):
                    i0 = ibh * 1024
                    for pair in range(PAIRS):
                        av_t = [ps_av.tile([128, 1024], FP, name=f"av{hh}", tag="av")
                                for hh in range(2)]
                        for jc in range(JC):
                            s_t = [ps_s.tile([128, 1024], FP, name=f"s{hh}", tag="s")
                                   for hh in range(2)]
                            for sb in range(2):
                                for hh in range(2):
                                    p0 = hh * 64
                                    nc.tensor.matmul(
                                        s_t[hh][:, sb * 512:(sb + 1) * 512],
                                        wTh[p0:p0 + 64, pair, jc * 128:(jc + 1) * 128],
                                        wTh[p0:p0 + 64, pair,
                                            i0 + sb * 512:i0 + (sb + 1) * 512],
                                        start=True, stop=True,
                                        tile_position=(p0, 0),
                                    )
                            e_t = []
                            for hh in range(2):
                                e = epool.tile([128, 1024], F16, name=f"e{hh}", tag="e")
                                # -10 logit shift: cancels in the softmax
                                # ratio but keeps exp within fp16 range (max
                                # observed logit ~19.8 = diag ||w||^2/8;
                                # fp16 overflows at e^11.09)
                                nc.scalar.activation(e[:], s_t[hh][:], EXPF,
                                                     scale=SCALE, bias=shiftb[:])
                                e_t.append(e)
                            for sb in range(2):
                                for hh in range(2):
                                    nc.tensor.matmul(
                                        av_t[hh][0:128, sb * 512:(sb + 1) * 512],
                                        v2[:, pair, jc, hh * 65:hh * 65 + 128],
                                        e_t[hh][:, sb * 512:(sb + 1) * 512],
                                        start=(jc == 0), stop=(jc == JC - 1),
                                        skip_group_check=True,
                                    )
                            # PE filler: previous i-half's output projection
                            # (also covers the pair-transition exp bubble)
                            if ibh == 1 and jc % 4 == 1:
                                emit_outproj(pair * 4 + (jc - 1) // 4)
                            # each block's first two jc iterations are a
                            # pipeline-refill transient (PE waits on exp in
                            # lockstep) -- pad them to full PE occupancy
                            if jc == 0:
                                emit_burst(ps_s, "s", 4)
                            elif jc == 1:
                                emit_burst(ps_s, "s", 3)

                        # ---- normalize this (ibh, pair): entirely PE-free.
                        # Denominator: row 64 of each AV accumulator -> SBUF
                        # -> [128, 8] spread (DMA iterates the out AP
                        # partition-major, i -> (i//8, i%8)) -> batched
                        # reciprocal -> DRAM bounce row -> partition-broadcast
                        # DMA into an SBUF [128, 1024] tile -> one in-place
                        # multiply on the staged raw AV.
                        kp = (ibh * 2 + pair) * 2
                        for hh in range(2):
                            k = kp + hh
                            trow = bsb.tile([1, 1024], FP, name="trow", tag="trow", bufs=4)
                            nc.vector.tensor_copy(trow[:], av_t[hh][64:65, :])
                            nc.sync.dma_start(den_sp[:, k * 8:(k + 1) * 8], trow[:])
                        nc.vector.tensor_copy(
                            osT2[0:64, pair, i0:i0 + 1024],
                            av_t[0][0:64, :],
                        )
                        nc.vector.tensor_copy(
                            osT2[64:128, pair, i0:i0 + 1024],
                            av_t[1][0:64, :],
                        )
                        nc.vector.reciprocal(recip_sp[:, kp * 8:(kp + 2) * 8],
                                             den_sp[:, kp * 8:(kp + 2) * 8])
                        bc = bsb.tile([128, 1024], FP, name="bc", tag="bc", bufs=2)
                        for hh in range(2):
                            k = kp + hh
                            nc.sync.dma_start(scr[k, :],
                                              recip_sp[:, k * 8:(k + 1) * 8])
                            nc.sync.dma_start(bc[hh * 64:(hh + 1) * 64, :],
                                              scr[k, :].partition_broadcast(64))
                        nc.vector.tensor_tensor(
                            out=osT2[:, pair, i0:i0 + 1024],
                            in0=osT2[:, pair, i0:i0 + 1024],
                            in1=bc[:],
                            op=mybir.AluOpType.mult,
                        )
                        # ride the PE through the dependency-thin block
                        # boundary (the next block's S stream is semaphore
                        # lockstepped with exp and leaves the PE ~70% idle
                        # for ~4us, which cold-traps the clock gate)
                        emit_burst(ps_s, "s", 12 if (ibh, pair) == (1, 1) else 4)

                # the second i-half's out-projection
                for ic in range(8, 16):
                    emit_outproj(ic)

    return nc


def get_program():
    if "nc" not in _program_cache:
        _program_cache["nc"] = build_program()
    return _program_cache["nc"]


def make_in_maps(x, mask, Wqkv, Wout):
    xT_b = [np.ascontiguousarray(x[b].T.astype(np.float16)) for b in range(2)]
    Wq16 = Wqkv.astype(np.float16)
    in_maps = []
    for c in range(8):
        b, hg = c // 4, c % 4
        ec = slice(hg * EC, (hg + 1) * EC)
        in_maps.append({
            "xT": xT_b[b],
            "wqkv": np.ascontiguousarray(Wq16[:, ec]),
            "wout": np.ascontiguousarray(Wout[ec, :].astype(np.float16)),
            "mask": np.ascontiguousarray(mask[b]),
        })
    return in_maps


def assemble(results, bout):
    y = np.stack([
        sum(results[b * 4 + g]["y"].astype(np.float32) for g in range(4))
        for b in range(2)
    ])
    return (y + bout[None, None, :]).astype(np.float32)


def kernel(x, mask, Wqkv, Wout, bout):
    _install_bir_legalizer()
    nc = get_program()
    in_maps = make_in_maps(x, mask, Wqkv, Wout)
    res = run_bass_kernel_spmd(nc, in_maps, core_ids=list(range(8)))
    return assemble(res.results, bout)


if __name__ == "__main__":
    nc = build_program()
    print("program built OK")
